# revision 1
# baseline (speedup 1.0000x reference)
"""Trainium2 Bass kernel: multi-head attention with 1x1-conv K/V projections,
per-head GhostBatchNorm (eval-mode affine), key+query masking, softmax.

Sharding: pure data parallelism over the batch axis (16 batches -> 8 cores,
2 per core).  No collectives.

Host-side mask compaction: the mask (1 = masked) removes each masked position
both as a KEY (softmax weight forced to 0) and as a QUERY (output row forced
to 0).  Since the K/V projections are 1x1 convs (per-position), masked
positions can be dropped on the host: per batch, gather the ~50% unmasked
positions of q/k_in/v_in into compact arrays padded to SPAD=640 columns, run
attention on the compact problem, then scatter the outputs back (zeros at
masked queries).  Padding columns carry a 0 "valid" flag which the kernel's
mask-column machinery uses to exclude them from softmax numerator and
denominator.  This cuts score/exp/PV work ~2.6x.

Per-core kernel (per batch), all big matmuls in float32r (single-pass
reduced-precision fp32; 4x PE throughput vs true fp32, ~2^-13 rel error):
  1. K projection  k[o,s] = sum_c k_w[o,c] k_in[c,s] + k_b[o]
     (host-transposed k_wT as lhsT; bias via per-partition tensor_scalar_add
      during the PSUM->SBUF copy).
  2. V projection TRANSPOSED vT[s,dv] (lhsT = v_in s-tile, rhs = v_wT; bias
     via rank-1 ones x v_b accumulate).  Copied into v_pv layout
     [p, chunk, head, 65]: 64 v columns zeroed at invalid (pad) positions
     plus a 65th column holding the valid flag, so the PV matmul produces
     numerator rows 0..63 and the softmax denominator in row 64.
  3. Scores TRANSPOSED sT[s,q] per head; dh=64, so the two heads of a pair
     run concurrently in the PE array via row tiling (base partitions 0/64).
     GBN scale is host-folded into q; the GBN bias is softmax-shift-invariant.
  4. E = exp(sT) on ScalarE from PSUM.  No max subtraction (scores bounded,
     fp32 exp cannot overflow for this problem's data).
  5. PV accumulates [65, QPAD] over the s-chunks.
  6. Epilogue per head: early PSUM->SBUF copy (frees the PSUM slot so the PE
     never stalls), 1/denominator via ACT Log + Exp(scale=-1) (the DVE
     reciprocal is ~6.5us for 640..1024 elements; ln+exp is ~2x0.8us),
     partition-broadcast of the scale row via a DRAM bounce (DMA reads the
     row 64x with a 0-stride partition AP), final multiply, DMA out.
"""

import numpy as np

BS, DA, SL, H = 16, 512, 1024, 8
N_CORES = 8
B = BS // N_CORES  # batches per core
P = 128
NT = DA // P       # channel tiles (4)
DH = DA // H       # head dim (64)

SPAD = 640         # padded compact sequence length (keys and queries)
NSP = SPAD // P    # compact s-chunks (5)
QPAD = SPAD

_CACHE: dict = {}


def build_nc(n_batches=B, n_pairs=H // 2):
    from contextlib import ExitStack

    import concourse.bass as bass
    import concourse.tile as tile
    from concourse import bacc, mybir

    dt = mybir.dt.float32
    dtr = mybir.dt.float32r
    bf16 = mybir.dt.bfloat16
    Alu = mybir.AluOpType
    Act = mybir.ActivationFunctionType

    nc = bacc.Bacc("TRN2", target_bir_lowering=False, debug=False)

    q_d = nc.dram_tensor("q", [n_batches, DA, SPAD], dtr, kind="ExternalInput")
    kin_d = nc.dram_tensor("k_in", [n_batches, DA, SPAD], dtr, kind="ExternalInput")
    vin_d = nc.dram_tensor("v_in", [n_batches, DA, SPAD], dtr, kind="ExternalInput")
    kwT_d = nc.dram_tensor("k_wT", [DA, DA], dtr, kind="ExternalInput")
    vwT_d = nc.dram_tensor("v_wT", [DA, DA], dtr, kind="ExternalInput")
    kb_d = nc.dram_tensor("k_b", [DA], dt, kind="ExternalInput")
    vb_d = nc.dram_tensor("v_b", [DA], dtr, kind="ExternalInput")
    ones_d = nc.dram_tensor("onesP", [P], dtr, kind="ExternalInput")
    mf_d = nc.dram_tensor("maskf", [n_batches, SPAD], dt, kind="ExternalInput")
    out_d = nc.dram_tensor("out", [n_batches, DA, QPAD], dt, kind="ExternalOutput")
    # DRAM bounce rows for the per-head scale broadcast
    scr_d = nc.dram_tensor("scale_bounce", [n_batches * H, QPAD], dt)

    NQ = [512, QPAD - 512]  # matmul N splits of the q free dim
    QO = [0, 512]

    with tile.TileContext(nc) as tc:
        with ExitStack() as ctx:
            consts = ctx.enter_context(tc.tile_pool(name="consts", bufs=1))
            qpool = ctx.enter_context(tc.tile_pool(name="qpool", bufs=2))
            kvpool = ctx.enter_context(tc.tile_pool(name="kvpool", bufs=1))
            kspool = ctx.enter_context(tc.tile_pool(name="kspool", bufs=2))
            vpvpool = ctx.enter_context(tc.tile_pool(name="vpvpool", bufs=2))
            mpool = ctx.enter_context(tc.tile_pool(name="mpool", bufs=2))
            epool = ctx.enter_context(tc.tile_pool(name="epool", bufs=3))
            opool = ctx.enter_context(tc.tile_pool(name="opool", bufs=4))
            orpool = ctx.enter_context(tc.tile_pool(name="orpool", bufs=8))
            scrpool = ctx.enter_context(tc.tile_pool(name="scrpool", bufs=8))
            bcpool = ctx.enter_context(tc.tile_pool(name="bcpool", bufs=4))
            psc = ctx.enter_context(tc.tile_pool(name="psc", bufs=2, space="PSUM"))
            ppv = ctx.enter_context(tc.tile_pool(name="ppv", bufs=2, space="PSUM"))

            # ---- constants ----
            kwT_sb = consts.tile([P, NT, DA], dtr)  # [p, ci, o]; c = ci*128+p
            nc.sync.dma_start(
                out=kwT_sb[:], in_=kwT_d.ap().rearrange("(ci p) o -> p ci o", p=P)
            )
            vwT_sb = consts.tile([P, NT, DA], dtr)
            nc.sync.dma_start(
                out=vwT_sb[:], in_=vwT_d.ap().rearrange("(ci p) o -> p ci o", p=P)
            )
            kb_col = consts.tile([P, NT], dt)  # k_b[o]; o = t*128+p
            nc.sync.dma_start(
                out=kb_col[:], in_=kb_d.ap().rearrange("(t p) -> p t", p=P)
            )
            vb_row = consts.tile([1, DA], dtr)
            nc.sync.dma_start(
                out=vb_row[:], in_=vb_d.ap().rearrange("(a o) -> a o", a=1)
            )
            ones_row = consts.tile([1, P], dtr)
            nc.sync.dma_start(
                out=ones_row[:], in_=ones_d.ap().rearrange("(a o) -> a o", a=1)
            )
            ones8 = consts.tile([P, H], dt)
            nc.vector.memset(ones8[:], 1.0)
            negC = consts.tile([P, 1], dt)
            nc.vector.memset(negC[:], -45.0)

            for b in range(n_batches):
                # ---- load inputs ----
                q_sb = qpool.tile([P, NT, SPAD], dtr)
                nc.sync.dma_start(
                    out=q_sb[:], in_=q_d.ap()[b].rearrange("(t p) s -> p t s", p=P)
                )
                kin_sb = kvpool.tile([P, NT, SPAD], dtr)
                nc.sync.dma_start(
                    out=kin_sb[:], in_=kin_d.ap()[b].rearrange("(t p) s -> p t s", p=P)
                )
                vin_sb = kvpool.tile([P, NT, SPAD], dtr)
                nc.sync.dma_start(
                    out=vin_sb[:], in_=vin_d.ap()[b].rearrange("(t p) s -> p t s", p=P)
                )
                maskf8 = mpool.tile([P, NSP], dt)  # valid flag, s = i*128+p
                nc.sync.dma_start(
                    out=maskf8[:], in_=mf_d.ap()[b].rearrange("(i p) -> p i", p=P)
                )

                # ---- K projection ----
                k_sb = kspool.tile([P, NT, SPAD], dtr)
                for t in range(NT):
                    kp = psc.tile([P, 640], dt, tag="sc", name="kp")
                    for ci in range(NT):
                        lhsT = kwT_sb[:, ci, t * P : (t + 1) * P]
                        for nh in range(2):
                            nc.tensor.matmul(
                                kp[:, QO[nh] : QO[nh] + NQ[nh]],
                                lhsT,
                                kin_sb[:, ci, QO[nh] : QO[nh] + NQ[nh]],
                                start=(ci == 0),
                                stop=(ci == NT - 1),
                            )
                    nc.vector.tensor_scalar_add(
                        k_sb[:, t, :], kp[:, :], kb_col[:, t : t + 1]
                    )

                # ---- V projection (transposed) + v_pv assembly ----
                v_pv = vpvpool.tile([P, NSP, H, DH + 1], bf16)
                for i in range(NSP):
                    vp = psc.tile([P, 640], dt, tag="sc", name="vp")[:, 0:DA]
                    for ci in range(NT):
                        nc.tensor.matmul(
                            vp[:, :],
                            vin_sb[:, ci, i * P : (i + 1) * P],
                            vwT_sb[:, ci, :],
                            start=(ci == 0),
                            stop=False,
                        )
                    nc.tensor.matmul(
                        vp[:, :], ones_row[:, :], vb_row[:, :], start=False, stop=True
                    )
                    nc.vector.tensor_scalar_mul(
                        v_pv[:, i, :, 0:DH],
                        vp[:].rearrange("p (h d) -> p h d", h=H),
                        maskf8[:, i : i + 1],
                    )
                    nc.vector.tensor_scalar_mul(
                        v_pv[:, i, :, DH], ones8[:, :], maskf8[:, i : i + 1]
                    )

                # ---- attention, head pairs ----
                pend = []  # deferred epilogues: (h, o_raw)
                for pr in range(n_pairs):
                    pvs = [
                        ppv.tile([P, 1024], dt, name=f"pv{j}", tag="pv")[:, 0:QPAD]
                        for j in range(2)
                    ]
                    for i in range(NSP):
                        scs = [
                            psc.tile([P, 640], dt, name=f"sc{j}", tag="sc")
                            for j in range(2)
                        ]
                        for hh in range(2):
                            lhsT = k_sb[
                                hh * 64 : (hh + 1) * 64, pr, i * P : (i + 1) * P
                            ]
                            for qo, nq in ((0, 512), (512, 128)):
                                nc.tensor.matmul(
                                    scs[hh][:, qo : qo + nq],
                                    lhsT,
                                    q_sb[
                                        hh * 64 : (hh + 1) * 64, pr, qo : qo + nq
                                    ],
                                    start=True,
                                    stop=True,
                                )
                        es = []
                        for hh in range(2):
                            e_sb = epool.tile([P, QPAD], bf16, name=f"e{hh}", tag="e")
                            # -45 shift keeps denominators inside the ACT Ln
                            # table range; softmax is shift-invariant.
                            nc.scalar.activation(
                                e_sb[:], scs[hh][:, :], Act.Exp, bias=negC[:, 0:1]
                            )
                            es.append(e_sb)
                        for hh in range(2):
                            lhsT = v_pv[:, i, 2 * pr + hh, :]
                            for qo, nq in ((0, 512), (512, 128)):
                                nc.tensor.matmul(
                                    pvs[hh][0:65, qo : qo + nq],
                                    lhsT,
                                    es[hh][:, qo : qo + nq],
                                    start=(i == 0),
                                    stop=(i == NSP - 1),
                                )
                    for hh in range(2):
                        h = 2 * pr + hh
                        o_raw = orpool.tile([65, QPAD], dt, name=f"oraw{h}", tag="oraw")
                        nc.vector.tensor_copy(o_raw[:, :], pvs[hh][0:65, :])
                        pend.append((h, o_raw))

                # ---- deferred epilogues (batched per ACT table set) ----
                scrs = {}
                for h, o_raw in pend:
                    scr = scrpool.tile([65, QPAD], dt, name=f"scr{h}", tag="scr")
                    nc.scalar.activation(scr[64:65, :], o_raw[64:65, :], Act.Ln)
                    scrs[h] = scr
                for h, o_raw in pend:
                    # 1/denom = exp(-ln(denom)); overwrite the consumed denom row
                    nc.scalar.activation(
                        o_raw[64:65, :], scrs[h][64:65, :], Act.Exp, scale=-1.0
                    )
                for h, o_raw in pend:
                    row = scr_d.ap()[b * H + h]
                    nc.sync.dma_start(out=row, in_=o_raw[64:65, :])
                    bc = bcpool.tile([64, QPAD], dt, name=f"bc{h}", tag="bc")
                    bcast_src = bass.AP(
                        tensor=row.tensor,
                        offset=row.offset,
                        ap=[[0, 64]] + list(row.ap),
                    )
                    nc.sync.dma_start(out=bc[:, :], in_=bcast_src)
                    o_sb = opool.tile([64, QPAD], dt, name=f"osb{h}", tag="osb")
                    nc.vector.tensor_tensor(
                        o_sb[:], o_raw[0:64, :], bc[:, :], op=Alu.mult
                    )
                    nc.sync.dma_start(
                        out=out_d.ap()[b, h * 64 : (h + 1) * 64, :], in_=o_sb[:]
                    )

    nc.compile()
    return nc


def _get_nc():
    if "nc" not in _CACHE:
        _CACHE["nc"] = build_nc()
    return _CACHE["nc"]


def _prepare(inputs):
    """Host-side compaction + sharding.  Returns (in_maps, keep_idx list)."""
    q = np.asarray(inputs["q"], dtype=np.float32)
    k_in = np.asarray(inputs["k_in"], dtype=np.float32)
    v_in = np.asarray(inputs["v_in"], dtype=np.float32)
    k_w = np.asarray(inputs["k_w"], dtype=np.float32)
    k_b = np.asarray(inputs["k_b"], dtype=np.float32)
    v_w = np.asarray(inputs["v_w"], dtype=np.float32)
    v_b = np.asarray(inputs["v_b"], dtype=np.float32)
    gamma = np.asarray(inputs["gbn_gamma"], dtype=np.float32)
    gs = np.asarray(inputs["gbn_s"], dtype=np.float32)
    mask = np.asarray(inputs["mask"]).reshape(BS, SL)

    # GBN affine: only the scale gamma/sd matters (additive part is
    # softmax-shift-invariant); fold into q per head.
    a = (gamma / gs).astype(np.float32)
    q_scaled = (
        (q.reshape(BS, H, DH, SL) * a[None, :, None, None]).reshape(BS, DA, SL)
    ).astype(np.float32)

    keeps = [np.flatnonzero(mask[b] == 0) for b in range(BS)]
    for b, kidx in enumerate(keeps):
        if len(kidx) > SPAD:
            raise ValueError(f"batch {b}: {len(kidx)} unmasked > SPAD={SPAD}")

    qc = np.zeros((BS, DA, SPAD), np.float32)
    kc = np.zeros((BS, DA, SPAD), np.float32)
    vc = np.zeros((BS, DA, SPAD), np.float32)
    mf = np.zeros((BS, SPAD), np.float32)
    for b, kidx in enumerate(keeps):
        n = len(kidx)
        qc[b, :, :n] = q_scaled[b][:, kidx]
        kc[b, :, :n] = k_in[b][:, kidx]
        vc[b, :, :n] = v_in[b][:, kidx]
        mf[b, :n] = 1.0

    k_wT = np.ascontiguousarray(k_w.T, dtype=np.float32)
    v_wT = np.ascontiguousarray(v_w.T, dtype=np.float32)
    onesP = np.ones(P, dtype=np.float32)

    in_maps = []
    for c in range(N_CORES):
        sl = slice(c * B, (c + 1) * B)
        in_maps.append(
            {
                "q": np.ascontiguousarray(qc[sl]),
                "k_in": np.ascontiguousarray(kc[sl]),
                "v_in": np.ascontiguousarray(vc[sl]),
                "k_wT": k_wT,
                "v_wT": v_wT,
                "k_b": k_b,
                "v_b": v_b.astype(np.float32),
                "onesP": onesP,
                "maskf": np.ascontiguousarray(mf[sl]),
            }
        )
    return in_maps, keeps


def _scatter(results, keeps) -> np.ndarray:
    out = np.zeros((BS, DA, SL), np.float32)
    for c in range(N_CORES):
        oc = results[c]["out"]  # [B, DA, QPAD]
        for bb in range(B):
            b = c * B + bb
            kidx = keeps[b]
            out[b][:, kidx] = oc[bb][:, : len(kidx)]
    return out


def kernel(**inputs) -> np.ndarray:
    from concourse.bass_utils import run_bass_kernel_spmd

    in_maps, keeps = _prepare(inputs)
    nc = _get_nc()
    res = run_bass_kernel_spmd(nc, in_maps, list(range(N_CORES)))
    return _scatter(res.results, keeps)



# revision 6
# speedup vs baseline: 1.6625x; 1.6625x over previous
"""Trainium2 Bass kernel: multi-head attention with 1x1-conv K/V projections,
per-head GhostBatchNorm (eval-mode affine), key+query masking, softmax.

Sharding: pure data parallelism over the batch axis (16 batches -> 8 cores),
with size-aware pairing: batches are sorted by unmasked count and each core
gets one small batch (compact length <= SPAD0, normally 512) and one large
batch (<= SPAD1, normally 544).  SPAD0 = 512 means the whole small batch runs
with single 512-wide PSUM pieces (no bank-split tail matmuls at all).

Host-side mask compaction (as v1): masked positions are dropped per batch,
kept positions gathered into compact arrays; a per-position valid flag
excludes pad columns from the softmax numerator/denominator via the v_pv
65th-column trick.

v2 changes vs the 278us/221us baseline:
  - fp16 for the whole q/k side (q, k_in, k_w, k_sb, scores matmul),
    bf16 for the v side (v_in, v_w, v_pv, E, PV matmul) and the output.
    Empirical pipeline sim: 0.67% max rel err (tolerance 2e-2).  Halves
    input DMA.  PE streams 1 row/cycle for both dtypes, but fp16/bf16
    allow the ldweights-reuse hack below and throttle less than fp32.
  - k_b dropped exactly: a key-bias shifts every score of a query column
    equally -> softmax invariant.  v_b folded exactly via the denominator:
    sum_s w (v+b) / sum_s w = PV/den + b, so v_pv = (v + b) * maskflag.
  - PSUM-piece tail matmuls (q columns 512..SPAD) set InstMatmult.ldweights
    = False and reuse the weights loaded by the 512-piece (verified on HW):
    small-N matmuls are otherwise paced by the ~225ns weight load.
  - Softmax reciprocal on DVE (reciprocal_approx_fast, ~51 ULP) instead of
    ACT Ln+Exp: kills the 42us of ACT_TABLE_LOAD thrash the tile scheduler
    caused by interleaving Ln/Exp with the main Exp stream.
  - Scores/exp/PV software-pipelined per head: PE order S(0) S(1) P(0)
    S(2) P(1) ... so exp(i) on ACT hides under scores(i+1).
  - Final normalize multiply on GpSimd (idle otherwise); o_raw PSUM->SBUF
    copy on ACT (Copy func, same Exp table, no swap).
  - 1/denominator partition-broadcast still via the DRAM-bounce DMA trick.
"""

import numpy as np

BS, DA, SL, H = 16, 512, 1024, 8
N_CORES = 8
B = BS // N_CORES  # batches per core (2)
P = 128
NT = DA // P       # channel tiles (4)
DH = DA // H       # head dim (64)

_CACHE: dict = {}


def _bucket(n: int) -> int:
    return max(128, -(-n // 32) * 32)


def _chunks(spad: int):
    cs = [(i * P, P) for i in range(spad // P)]
    if spad % P:
        cs.append((spad // P * P, spad % P))
    return cs


def _pieces(spad: int):
    if spad <= 512:
        return [(0, spad)]
    return [(0, 512), (512, spad - 512)]


def build_nc(spads):
    from contextlib import ExitStack

    import concourse.bass as bass
    import concourse.tile as tile
    from concourse import bacc, mybir

    f32 = mybir.dt.float32
    f16 = mybir.dt.float16
    bf16 = mybir.dt.bfloat16
    Alu = mybir.AluOpType
    Act = mybir.ActivationFunctionType

    nc = bacc.Bacc("TRN2", target_bir_lowering=False, debug=False)

    q_d, kin_d, vin_d, mf_d, out_d, scr_d = [], [], [], [], [], []
    for b, spad in enumerate(spads):
        nch = len(_chunks(spad))
        q_d.append(nc.dram_tensor(f"q{b}", [DA, spad], f16, kind="ExternalInput"))
        kin_d.append(nc.dram_tensor(f"kin{b}", [DA, spad], f16, kind="ExternalInput"))
        vin_d.append(nc.dram_tensor(f"vin{b}", [DA, spad], bf16, kind="ExternalInput"))
        mf_d.append(nc.dram_tensor(f"mf{b}", [nch * P], f32, kind="ExternalInput"))
        out_d.append(nc.dram_tensor(f"out{b}", [DA, spad], bf16, kind="ExternalOutput"))
        scr_d.append(nc.dram_tensor(f"scr{b}", [H, spad], f32))
    kwT_d = nc.dram_tensor("kwT", [DA, DA], f16, kind="ExternalInput")
    vwT_d = nc.dram_tensor("vwT", [DA, DA], bf16, kind="ExternalInput")
    vb_d = nc.dram_tensor("vb", [1, DA], f32, kind="ExternalInput")

    with tile.TileContext(nc) as tc:
        with ExitStack() as ctx:
            consts = ctx.enter_context(tc.tile_pool(name="consts", bufs=1))
            inpool = ctx.enter_context(tc.tile_pool(name="inpool", bufs=2))
            kspool = ctx.enter_context(tc.tile_pool(name="kspool", bufs=2))
            vpvpool = ctx.enter_context(tc.tile_pool(name="vpvpool", bufs=2))
            vtpool = ctx.enter_context(tc.tile_pool(name="vtpool", bufs=2))
            epool = ctx.enter_context(tc.tile_pool(name="epool", bufs=3))
            orpool = ctx.enter_context(tc.tile_pool(name="orpool", bufs=3))
            scrpool = ctx.enter_context(tc.tile_pool(name="scrpool", bufs=3))
            bcpool = ctx.enter_context(tc.tile_pool(name="bcpool", bufs=3))
            opool = ctx.enter_context(tc.tile_pool(name="opool", bufs=3))
            psc = ctx.enter_context(tc.tile_pool(name="psc", bufs=2, space="PSUM"))
            ppv = ctx.enter_context(tc.tile_pool(name="ppv", bufs=2, space="PSUM"))

            # ---- constants ----
            kwT_sb = consts.tile([P, NT, DA], f16)  # [p, ci, o]; c = ci*128+p
            nc.sync.dma_start(
                out=kwT_sb[:], in_=kwT_d.ap().rearrange("(ci p) o -> p ci o", p=P)
            )
            vwT_sb = consts.tile([P, NT, DA], bf16)
            nc.sync.dma_start(
                out=vwT_sb[:], in_=vwT_d.ap().rearrange("(ci p) o -> p ci o", p=P)
            )
            vb_bc = consts.tile([P, DA], f32)  # v_b broadcast to all partitions
            vb_row = vb_d.ap()[0]
            nc.sync.dma_start(
                out=vb_bc[:],
                in_=bass.AP(
                    tensor=vb_row.tensor,
                    offset=vb_row.offset,
                    ap=[[0, P]] + list(vb_row.ap),
                ),
            )
            ones8 = consts.tile([P, H], f32)
            nc.vector.memset(ones8[:], 1.0)
            negC = consts.tile([P, 1], f32)
            nc.vector.memset(negC[:], -45.0)

            for b, spad in enumerate(spads):
                cs = _chunks(spad)
                qps = _pieces(spad)
                ncs = len(cs)

                # ---- input loads ----
                kin_sb = inpool.tile([P, NT, spad], f16, tag="kin", name=f"kin_{b}")
                nc.sync.dma_start(
                    out=kin_sb[:], in_=kin_d[b].ap().rearrange("(t p) s -> p t s", p=P)
                )
                q_sb = inpool.tile([P, NT, spad], f16, tag="q", name=f"q_{b}")
                nc.sync.dma_start(
                    out=q_sb[:], in_=q_d[b].ap().rearrange("(t p) s -> p t s", p=P)
                )
                vin_sb = inpool.tile([P, NT, spad], bf16, tag="vin", name=f"vin_{b}")
                nc.sync.dma_start(
                    out=vin_sb[:], in_=vin_d[b].ap().rearrange("(t p) s -> p t s", p=P)
                )
                maskf = inpool.tile([P, ncs], f32, tag="mf", name=f"mf_{b}")
                nc.sync.dma_start(
                    out=maskf[:], in_=mf_d[b].ap().rearrange("(i p) -> p i", p=P)
                )

                # ---- K projection (no bias; softmax shift-invariant) ----
                k_sb = kspool.tile([P, NT, spad], f16, tag="k", name=f"k_{b}")
                for t in range(NT):
                    kp = psc.tile([P, spad], f32, tag="sc", name=f"kp{t}")
                    for ci in range(NT):
                        lhsT = kwT_sb[:, ci, t * P : (t + 1) * P]
                        for qo, nq in qps:
                            nc.tensor.matmul(
                                kp[:, qo : qo + nq],
                                lhsT,
                                kin_sb[:, ci, qo : qo + nq],
                                start=(ci == 0),
                                stop=(ci == NT - 1),
                            )
                    nc.vector.tensor_copy(k_sb[:, t, :], kp[:, :])

                # ---- V projection (transposed) + v_pv assembly ----
                v_pv = vpvpool.tile([P, ncs, H, DH + 1], bf16, tag="vpv", name=f"vpv_{b}")
                for i, (s0, sc) in enumerate(cs):
                    vp = psc.tile([P, spad], f32, tag="sc", name=f"vp{i}")
                    for ci in range(NT):
                        nc.tensor.matmul(
                            vp[0:sc, 0:DA],
                            vin_sb[:, ci, s0 : s0 + sc],
                            vwT_sb[:, ci, :],
                            start=(ci == 0),
                            stop=(ci == NT - 1),
                        )
                    vt = vtpool.tile([P, DA], bf16, tag="vt", name=f"vt_{b}_{i}")
                    nc.vector.tensor_tensor(
                        vt[0:sc, :], vp[0:sc, 0:DA], vb_bc[0:sc, :], op=Alu.add
                    )
                    nc.vector.tensor_scalar_mul(
                        v_pv[0:sc, i, :, 0:DH],
                        vt[0:sc].rearrange("p (h d) -> p h d", h=H),
                        maskf[0:sc, i : i + 1],
                    )
                    nc.vector.tensor_scalar_mul(
                        v_pv[0:sc, i, :, DH], ones8[0:sc, :], maskf[0:sc, i : i + 1]
                    )

                # ---- attention, one head at a time, chunk-pipelined ----
                for h in range(H):
                    t, pb = h // 2, (h % 2) * DH
                    scs_tiles = {}

                    def emit_scores(i):
                        s0, sc = cs[i]
                        scs = psc.tile([P, spad], f32, tag="sc", name=f"sc{h}_{i}")
                        lhsT = k_sb[pb : pb + DH, t, s0 : s0 + sc]
                        for qo, nq in qps:
                            nc.tensor.matmul(
                                scs[0:sc, qo : qo + nq],
                                lhsT,
                                q_sb[pb : pb + DH, t, qo : qo + nq],
                                start=True,
                                stop=True,
                            )
                        scs_tiles[i] = scs

                    emit_scores(0)
                    pv = ppv.tile([DH + 1, spad], f32, tag="pv", name=f"pv{h}")
                    for i, (s0, sc) in enumerate(cs):
                        e_sb = epool.tile([P, spad], bf16, tag="e", name=f"e{h}_{i}")
                        # -45 shift keeps exp in fp32/bf16 range; softmax is
                        # shift-invariant.
                        nc.scalar.activation(
                            e_sb[0:sc, :],
                            scs_tiles.pop(i)[0:sc, :],
                            Act.Exp,
                            bias=negC[0:sc, 0:1],
                        )
                        if i + 1 < ncs:
                            emit_scores(i + 1)
                        lhsT = v_pv[0:sc, i, h, :]
                        for qo, nq in qps:
                            nc.tensor.matmul(
                                pv[0 : DH + 1, qo : qo + nq],
                                lhsT,
                                e_sb[0:sc, qo : qo + nq],
                                start=(i == 0),
                                stop=(i == ncs - 1),
                            )

                    # ---- epilogue: normalize by the accumulated denominator ----
                    # reciprocal_approx_fast is a custom-DVE op whose reads and
                    # writes are INVISIBLE to the tile dependency tracker: it
                    # must be sandwiched between normal vector-engine ops (same
                    # queue -> program order) that carry the real dependencies.
                    o_raw = orpool.tile([DH + 1, spad], f32, tag="oraw", name=f"or{h}")
                    nc.scalar.activation(o_raw[:, :], pv[0 : DH + 1, :], Act.Copy)
                    den = scrpool.tile([1, spad], f32, tag="den", name=f"den{h}")
                    nc.vector.tensor_copy(den[0:1, :], pv[DH : DH + 1, :])
                    rec = scrpool.tile([1, spad], f32, tag="rec", name=f"rec{h}")
                    nc.vector.reciprocal_approx_fast(out=rec[0:1, :], in_=den[0:1, :])
                    rec2 = scrpool.tile([1, spad], f32, tag="rec2", name=f"rec2{h}")
                    nc.vector.tensor_copy(rec2[0:1, :], rec[0:1, :])
                    row = scr_d[b].ap()[h]
                    nc.sync.dma_start(out=row, in_=rec2[0:1, :])
                    bc = bcpool.tile([DH, spad], f32, tag="bc", name=f"bc{h}")
                    nc.sync.dma_start(
                        out=bc[:, :],
                        in_=bass.AP(
                            tensor=row.tensor,
                            offset=row.offset,
                            ap=[[0, DH]] + list(row.ap),
                        ),
                    )
                    o_sb = opool.tile([DH, spad], bf16, tag="osb", name=f"osb{h}")
                    nc.gpsimd.tensor_tensor(
                        o_sb[:, :], o_raw[0:DH, :], bc[:, :], op=Alu.mult
                    )
                    nc.sync.dma_start(
                        out=out_d[b].ap()[h * DH : (h + 1) * DH, :], in_=o_sb[:, :]
                    )

    nc.compile()
    return nc


def _get_nc(spads):
    key = tuple(spads)
    if key not in _CACHE:
        _CACHE[key] = build_nc(key)
    return _CACHE[key]


def _prepare(inputs):
    """Mask compaction, GBN folding, size-aware batch pairing, sharding."""
    import ml_dtypes

    q = np.asarray(inputs["q"], dtype=np.float32)
    k_in = np.asarray(inputs["k_in"], dtype=np.float32)
    v_in = np.asarray(inputs["v_in"], dtype=np.float32)
    k_w = np.asarray(inputs["k_w"], dtype=np.float32)
    v_w = np.asarray(inputs["v_w"], dtype=np.float32)
    v_b = np.asarray(inputs["v_b"], dtype=np.float32)
    gamma = np.asarray(inputs["gbn_gamma"], dtype=np.float32)
    gs = np.asarray(inputs["gbn_s"], dtype=np.float32)
    mask = np.asarray(inputs["mask"]).reshape(BS, SL)

    # GBN affine: only gamma/sd matters (additive part and k_b are softmax
    # shift-invariant); fold the scale into q per head.
    a = (gamma / gs).astype(np.float32)
    q_scaled = (
        (q.reshape(BS, H, DH, SL) * a[None, :, None, None]).reshape(BS, DA, SL)
    ).astype(np.float32)

    keeps = [np.flatnonzero(mask[b] == 0) for b in range(BS)]
    counts = np.array([len(k) for k in keeps])
    order = np.argsort(counts, kind="stable")
    # large batches first: the kernel tail (last head's epilogue chain) then
    # belongs to the cheaper small batch.
    slots = [order[N_CORES:], order[:N_CORES]]
    spads = tuple(
        _bucket(int(counts[sl].max()) if len(sl) else 128) for sl in slots
    )

    k_wT = np.ascontiguousarray(k_w.T).astype(np.float16)
    v_wT = np.ascontiguousarray(v_w.T).astype(ml_dtypes.bfloat16)
    vb2 = v_b.reshape(1, DA).astype(np.float32)

    in_maps = [
        {"kwT": k_wT, "vwT": v_wT, "vb": vb2} for _ in range(N_CORES)
    ]
    assign = np.zeros((N_CORES, B), dtype=np.int64)
    for s, (sl, spad) in enumerate(zip(slots, spads)):
        nch = len(_chunks(spad))
        for c in range(N_CORES):
            gb = int(sl[c])
            assign[c, s] = gb
            kidx = keeps[gb]
            n = len(kidx)
            qc = np.zeros((DA, spad), np.float16)
            kc = np.zeros((DA, spad), np.float16)
            vc = np.zeros((DA, spad), ml_dtypes.bfloat16)
            mf = np.zeros((nch * P,), np.float32)
            qc[:, :n] = q_scaled[gb][:, kidx].astype(np.float16)
            kc[:, :n] = k_in[gb][:, kidx].astype(np.float16)
            vc[:, :n] = v_in[gb][:, kidx].astype(ml_dtypes.bfloat16)
            mf[:n] = 1.0
            in_maps[c][f"q{s}"] = qc
            in_maps[c][f"kin{s}"] = kc
            in_maps[c][f"vin{s}"] = vc
            in_maps[c][f"mf{s}"] = mf
    return in_maps, keeps, assign, spads


def _scatter(results, keeps, assign) -> np.ndarray:
    out = np.zeros((BS, DA, SL), np.float32)
    for c in range(N_CORES):
        for s in range(B):
            gb = int(assign[c, s])
            kidx = keeps[gb]
            oc = np.asarray(results[c][f"out{s}"]).astype(np.float32)
            out[gb][:, kidx] = oc[:, : len(kidx)]
    return out


def kernel(**inputs) -> np.ndarray:
    from concourse.bass_utils import run_bass_kernel_spmd

    in_maps, keeps, assign, spads = _prepare(inputs)
    nc = _get_nc(spads)
    res = run_bass_kernel_spmd(nc, in_maps, list(range(N_CORES)))
    return _scatter(res.results, keeps, assign)
